# revision 29
# baseline (speedup 1.0000x reference)
"""AdaptiveWeightInterpolationModule on 8 Trainium2 NeuronCores, v2.

Forward-splat frame interpolation: two branches, each = bilinear backward
warp (photometric weight fw) + truncated-gaussian forward splat of
(pixel*fw, fw, ones), then adaptive blend.

Strategy v2 (vs baseline)
-------------------------
- Flow-range clipping: pixels whose flow magnitude >= S (=2.0) on either
  axis are outliers (~9% for randn flow).  The device computes the dense
  masked-shift warp/splat only over the small clipped range (splat pairs
  167->~49, warp pairs 115->~25, and halos SH 7->3 so every tile shrinks
  ~1.4x).  Outliers are handled exactly:
    * warp: device samples with flow 0 (identity, weight 1) and adds a
      host-computed correction plane (true bilinear sample - center).
    * splat: outlier sources are dropped on device (PADFLOW) and their
      16-tap contributions are added on the host into the accumulator
      planes, using the device-computed fw plane (exact).
  The device outputs the 5 accumulator planes per branch + fw planes;
  the host applies the fixup and the final (cheap) blend.
- Single q-threshold mask (is_lt on the squared distance) instead of
  is_ge*is_lt per tap axis.
- fp16 4B-alignment: +1-column shifted duplicates of the shifted-read
  tiles (Fb, i2b, wX, wY) so every pair MAC runs in the DVE 2x perf mode
  regardless of shift parity.
- Patch layout as baseline: 128 partitions each own a 30x40 out-tile,
  free dim = flattened patch with halos; data-dependent taps are dense
  masked shift-mul-accs; weights on the Scalar engine.
"""

import numpy as np

import concourse.bass as bass
from concourse import bacc, mybir
from concourse.tile import TileContext
from concourse import bass_utils

F32 = mybir.dt.float32
FP16 = mybir.dt.float16
AF = mybir.ActivationFunctionType
OP = mybir.AluOpType

SIGMA_D = 1.5
TAO_R = 0.05
LAMBDA_E = 30.0 / 255.0
THRESH = 1e-6
EPS = 1e-6
PADFLOW = 30000.0
GSC = 1.0 / (SIGMA_D * np.sqrt(2.0))
QTHR = 4.0 * GSC * GSC          # window mask threshold on ((d)*gsc)^2
INV2S2 = 1.0 / (2.0 * SIGMA_D * SIGMA_D)

_PROGRAM_CACHE = {}


class Geo:
    def __init__(self, H, W, NSY, NSX, npart, SH, WH):
        self.H, self.W = H, W
        self.NSY, self.NSX = NSY, NSX
        assert W % NSX == 0
        self.OC = W // NSX
        self.npart = npart
        assert (NSY * NSX) % npart == 0
        self.npass = (NSY * NSX) // npart
        assert npart % NSX == 0
        self.SYPP = npart // NSX
        self.SH = SH
        self.EH = SH + 1
        self.WH = WH
        self.IH = self.EH + WH

    def finish(self, H2):
        assert H2 % self.NSY == 0
        self.H2 = H2
        self.OR = H2 // self.NSY
        self.PR = self.OR + 2 * self.IH
        self.PC = self.OC + 2 * self.IH
        self.ER = self.OR + 2 * self.EH
        self.EC = self.OC + 2 * self.EH
        self.SR = self.OR + 2 * self.SH
        self.SC = self.OC + 2 * self.SH
        self.Rpad = H2 + 2 * self.IH
        self.Cpad = self.W + 2 * self.IH
        return self

    def key(self):
        return (self.H, self.W, self.NSY, self.NSX, self.npart, self.SH,
                self.WH, self.H2)


def _splat_pairs(flow_list, sh):
    occ = set()
    for fl in flow_list:
        H, W = fl.shape[1], fl.shape[2]
        ys = np.arange(H, dtype=np.float32)[:, None]
        xs = np.arange(W, dtype=np.float32)[None, :]
        ueff = (xs + fl[0]) - xs
        veff = (ys + fl[1]) - ys
        ku = np.floor(ueff).astype(np.int64).ravel() + 4096
        kv = np.floor(veff).astype(np.int64).ravel() + 4096
        occ.update(np.unique(kv * 8192 + ku).tolist())
    pairs = set()
    for o in occ:
        a, b = divmod(o, 8192)
        a -= 4096
        b -= 4096
        for d in (-1, 0, 1, 2):
            for e in (-1, 0, 1, 2):
                pairs.add((a + d, b + e))
    pairs = sorted(pairs)
    for (dy, dx) in pairs:
        assert abs(dy) <= sh and abs(dx) <= sh, (dy, dx, sh)
    return pairs


def _warp_pairs(flow_list, H, W, wh):
    occ = set()
    for fl in flow_list:
        u, v = fl[0], fl[1]
        ys = np.arange(H, dtype=np.float32)[:, None]
        xs = np.arange(W, dtype=np.float32)[None, :]
        kv = np.floor(np.clip(ys + v, 0.0, H - 1.0) - ys).astype(np.int64).ravel() + 4096
        ku = np.floor(np.clip(xs + u, 0.0, W - 1.0) - xs).astype(np.int64).ravel() + 4096
        occ.update(np.unique(kv * 8192 + ku).tolist())
    pairs = set()
    for o in occ:
        a, b = divmod(o, 8192)
        a -= 4096
        b -= 4096
        for d in (0, 1):
            for e in (0, 1):
                pairs.add((a + d, b + e))
    pairs = sorted(pairs)
    for (ky, kx) in pairs:
        assert abs(ky) <= wh and abs(kx) <= wh, (ky, kx, wh)
    return pairs


def _bc(ap, n):
    """Insert a broadcast (step-0) dim of size n after the partition dim."""
    return bass.AP(ap.tensor, ap.offset, [ap.ap[0], [0, n]] + list(ap.ap[1:]))


def _build_program(geo, splat_pairs, warp_pairs, num_devices):
    g = geo
    nc = bacc.Bacc("TRN2", target_bir_lowering=False, debug=False,
                   num_devices=num_devices)

    # pre-register activation bias constants
    biases = {1.0}
    for wpr_ in warp_pairs:
        for (ky, kx) in wpr_:
            biases.add(float(-ky))
            biases.add(float(-kx))
    for spr_ in splat_pairs:
        for (dy, dx) in spr_:
            biases.add(float(-dy) * GSC)
            biases.add(float(-dx) * GSC)
    for i, val in enumerate(sorted(biases)):
        key = (F32, val)
        if key not in nc.const_aps.aps:
            t = nc.alloc_sbuf_tensor(f"constap{i}", [128, 1], F32)
            nc.gpsimd.memset(t.ap(), val)
            nc.const_aps.aps[key] = t.ap()
    nc.all_engine_barrier()

    dr = {}
    for nm in ("i1", "i2"):
        dr[nm] = nc.dram_tensor(nm, [3, g.Rpad, g.Cpad], FP16, kind="ExternalInput").ap()
    for nm in ("f1w", "f2w", "f1s", "f2s"):
        dr[nm] = nc.dram_tensor(nm, [2, g.Rpad, g.Cpad], F32, kind="ExternalInput").ap()
    for nm in ("c1", "c2"):
        dr[nm] = nc.dram_tensor(nm, [3, g.Rpad, g.Cpad], FP16, kind="ExternalInput").ap()
    for nm in ("vm",):
        dr[nm] = nc.dram_tensor(nm, [g.Rpad, g.Cpad], F32, kind="ExternalInput").ap()
    dr["ident"] = nc.dram_tensor("ident", [128, 128], FP16,
                                 kind="ExternalInput").ap()
    outs = {
        "a1": nc.dram_tensor("a1", [5, g.H2, g.W], FP16, kind="ExternalOutput").ap(),
        "a2": nc.dram_tensor("a2", [5, g.H2, g.W], FP16, kind="ExternalOutput").ap(),
        "fw1": nc.dram_tensor("fw1", [g.H2, g.W], FP16, kind="ExternalOutput").ap(),
        "fw2": nc.dram_tensor("fw2", [g.H2, g.W], FP16, kind="ExternalOutput").ap(),
    }

    P = g.npart

    def load_region(tile_ap, base2d, pass_i, rh):
        nr = g.OR + 2 * rh
        nc_ = g.OC + 2 * rh
        off = (base2d.offset
               + (pass_i * g.SYPP * g.OR + g.IH - rh) * g.Cpad
               + (g.IH - rh))
        for sy in range(g.SYPP):
            ap = bass.AP(base2d.tensor, off + sy * g.OR * g.Cpad,
                         [[g.OC, g.NSX], [g.Cpad, nr], [1, nc_]])
            nc.sync.dma_start(out=tile_ap[sy * g.NSX:(sy + 1) * g.NSX], in_=ap)

    def store_plane(tile_ap, base2d, pass_i):
        off = base2d.offset + pass_i * g.SYPP * g.OR * g.W
        for sy in range(g.SYPP):
            ap = bass.AP(base2d.tensor, off + sy * g.OR * g.W,
                         [[g.OC, g.NSX], [g.W, g.OR], [1, g.OC]])
            nc.sync.dma_start(out=ap, in_=tile_ap[sy * g.NSX:(sy + 1) * g.NSX])

    tt = nc.vector.tensor_tensor
    tss = nc.vector.tensor_single_scalar
    act = nc.scalar.activation

    def mk_odd(pool, src, shape, tag):
        """+1-column copy so odd free-dim shifts read 4B-aligned."""
        t = pool.tile(shape, FP16, tag=tag, name=tag)
        sl_in = tuple([slice(None)] * (len(shape) - 1) + [slice(1, shape[-1])])
        sl_out = tuple([slice(None)] * (len(shape) - 1) + [slice(0, shape[-1] - 1)])
        nc.vector.tensor_copy(out=t[sl_out], in_=src[sl_in])
        return t

    def flat_ap(ap, s, n):
        return bass.AP(ap.tensor, ap.offset + s, [ap.ap[0], [1, n]])

    # ragged bank-aligned PSUM chunking for nfp32 contiguous accumulators
    def psum_chunks(ntot):
        chunks = []
        flat = 0
        while flat < ntot:
            room = 512 - (flat % 512)
            sz = min(512, ntot - flat, room)
            chunks.append((flat, sz, flat % 512 == 0))
            flat += sz
        return chunks

    NP3 = 3 * 30 * 40  # 3 fp32 accumulator channels, must fit 8 PSUM banks
    assert 3 * g.OR * g.OC == NP3 and NP3 <= 4096

    with TileContext(nc) as tc:
      with tc.tile_pool(name="sing", bufs=1) as sing:
        identt = sing.tile([128, 128], FP16, tag="id")
        nc.sync.dma_start(out=identt[:], in_=dr["ident"])
        for pass_i in range(g.npass):
            for br in range(2):
                imA = dr["i1"] if br == 0 else dr["i2"]
                imB = dr["i2"] if br == 0 else dr["i1"]
                flw = dr["f1w"] if br == 0 else dr["f2w"]
                fls = dr["f1s"] if br == 0 else dr["f2s"]
                crr = dr["c1"] if br == 0 else dr["c2"]
                aout = outs["a1"] if br == 0 else outs["a2"]
                fwout = outs["fw1"] if br == 0 else outs["fw2"]
                spr = splat_pairs[br]
                wpr = warp_pairs[br]

                with tc.tile_pool(name="brp", bufs=1) as brp:
                    errt = brp.tile([P, g.ER, g.EC], F32, tag="err")
                    i1b = brp.tile([P, 3, g.ER, g.EC], FP16, tag="i1b")
                    acc5 = brp.tile([P, 5, g.OR, g.OC], FP16, tag="acc5")

                    # ================= warp + err =================
                    with tc.tile_pool(name="wp", bufs=1) as wp:
                        for c in range(3):
                            load_region(i1b[:, c], imA[c], pass_i, g.EH)
                        vp = wp.tile([P, g.ER, g.EC], F32, tag="vp")
                        up = wp.tile([P, g.ER, g.EC], F32, tag="up")
                        ct = wp.tile([P, g.ER, g.EC], F32, tag="ct")
                        load_region(vp[:], flw[1], pass_i, g.EH)
                        load_region(up[:], flw[0], pass_i, g.EH)
                        load_region(ct[:], dr["vm"], pass_i, g.EH)

                        i2b = wp.tile([P, 3, g.PR, g.PC], FP16, tag="i2b")
                        for c in range(3):
                            load_region(i2b[:, c], imB[c], pass_i, g.IH)
                        i2bo = mk_odd(wp, i2b, [P, 3, g.PR, g.PC], "i2bo")

                        # per-kx / per-ky triangle weights (Scalar engine)
                        kxs = sorted({kx for (_, kx) in wpr})
                        kys = sorted({ky for (ky, _) in wpr})
                        # interleaved emission: the first warp pair only needs
                        # triy[kys[0]] and trix[kxs[0]], so DVE unblocks after
                        # 4 activations instead of the full set
                        trix, triy = {}, {}
                        for i in range(max(len(kxs), len(kys))):
                            if i < len(kys):
                                ky = kys[i]
                                t = wp.tile([P, g.ER, g.EC], FP16,
                                            tag=f"triy{ky}", name=f"triy{ky}")
                                act(out=t[:], in_=vp[:], func=AF.Abs,
                                    bias=float(-ky))
                                act(out=t[:], in_=t[:], func=AF.Relu,
                                    scale=-1.0, bias=1.0)
                                triy[ky] = t
                            if i < len(kxs):
                                kx = kxs[i]
                                t = wp.tile([P, g.ER, g.EC], FP16,
                                            tag=f"trix{kx}", name=f"trix{kx}")
                                act(out=t[:], in_=up[:], func=AF.Abs,
                                    bias=float(-kx))
                                act(out=t[:], in_=t[:], func=AF.Relu,
                                    scale=-1.0, bias=1.0)
                                trix[kx] = t

                        wacc = wp.tile([P, 3, g.ER, g.EC], FP16, tag="wacc")
                        # corr planes seed the accumulator (outlier fixup);
                        # staged copy: never DMA into a tile that is then RMW'd
                        ldh = wp.tile([P, 3, g.ER, g.EC], FP16, tag="ldh")
                        for c in range(3):
                            load_region(ldh[:, c], crr[c], pass_i, g.EH)
                        nc.vector.tensor_copy(out=wacc[:, 2], in_=ldh[:, 2])

                        NPW = 2 * g.ER * g.EC
                        assert NPW <= 4096
                        CHW = psum_chunks(NPW)
                        d0 = g.IH - g.EH
                        with tc.tile_pool(name="wpp", bufs=1, space="PSUM") as wpp:
                            psw = wpp.tile([P, NPW], F32, tag="pw")
                            # seed PSUM chs 0-1 with the corr planes
                            l2 = ldh[:]
                            for (s, sz, bstart) in CHW:
                                nc.tensor.matmul(out=psw[:, s:s + sz],
                                                 lhsT=identt[:],
                                                 rhs=flat_ap(l2, s, sz),
                                                 start=bstart, stop=False)
                            nwpr = len(wpr)
                            for pi, (ky, kx) in enumerate(wpr):
                                wpair = wp.tile([P, g.ER, g.EC], FP16, tag="wpair")
                                tt(out=wpair[:], in0=triy[ky][:], in1=trix[kx][:],
                                   op=OP.mult)
                                oc = d0 + kx
                                src = i2b if oc % 2 == 0 else i2bo
                                if oc % 2 != 0:
                                    oc -= 1
                                tmp3 = wp.tile([P, 3, g.ER, g.EC], FP16,
                                               tag=f"tmp3{pi % 3}",
                                               name=f"tmp3{pi % 3}")
                                tt(out=tmp3[:],
                                   in0=src[:, :, d0 + ky:d0 + ky + g.ER, oc:oc + g.EC],
                                   in1=_bc(wpair[:], 3), op=OP.mult)
                                t3 = tmp3[:]
                                for (s, sz, bstart) in CHW:
                                    nc.tensor.matmul(out=psw[:, s:s + sz],
                                                     lhsT=identt[:],
                                                     rhs=flat_ap(t3, s, sz),
                                                     start=False,
                                                     stop=(pi == nwpr - 1))
                                tt(out=wacc[:, 2], in0=wacc[:, 2],
                                   in1=tmp3[:, 2], op=OP.add)
                            act(out=wacc[:, 0:2], in_=psw[:, 0:NPW],
                                func=AF.Copy)

                        # err = (sum_c |i1_c - wacc_c|) * vmask
                        d3 = wp.tile([P, 3, g.ER, g.EC], FP16, tag="tmp30")
                        tt(out=d3[:], in0=i1b[:], in1=wacc[:], op=OP.subtract)
                        act(out=d3[:], in_=d3[:], func=AF.Abs)
                        tt(out=errt[:], in0=d3[:, 0], in1=d3[:, 1], op=OP.add)
                        tt(out=errt[:], in0=errt[:], in1=d3[:, 2], op=OP.add)
                        tt(out=errt[:], in0=errt[:], in1=ct[:], op=OP.mult)

                    # ================= blur -> fw -> F =================
                    Fb = brp.tile([P, 4, g.SR, g.SC], FP16, tag="Fb")
                    with tc.tile_pool(name="bp", bufs=1) as bp:
                        d1 = g.EH - g.SH  # = 1
                        tmpb = bp.tile([P, g.ER, g.SC], F32, tag="tmpb")
                        tt(out=tmpb[:], in0=errt[:, :, d1 - 1:d1 - 1 + g.SC],
                           in1=errt[:, :, d1 + 1:d1 + 1 + g.SC], op=OP.add)
                        tt(out=tmpb[:], in0=tmpb[:],
                           in1=errt[:, :, d1:d1 + g.SC], op=OP.add)
                        blur = bp.tile([P, g.SR, g.SC], F32, tag="blur")
                        tt(out=blur[:], in0=tmpb[:, d1 - 1:d1 - 1 + g.SR, :],
                           in1=tmpb[:, d1 + 1:d1 + 1 + g.SR, :], op=OP.add)
                        tt(out=blur[:], in0=blur[:],
                           in1=tmpb[:, d1:d1 + g.SR, :], op=OP.add)
                        s = 1.0 / (27.0 * LAMBDA_E)
                        act(out=blur[:], in_=blur[:], func=AF.Square, scale=float(s))
                        act(out=Fb[:, 3], in_=blur[:], func=AF.Exp, scale=-1.0)
                        for c in range(3):
                            tt(out=Fb[:, c], in0=i1b[:, c, d1:d1 + g.SR, d1:d1 + g.SC],
                               in1=Fb[:, 3], op=OP.mult)
                    store_plane(Fb[:, 3, g.SH:g.SH + g.OR, g.SH:g.SH + g.OC],
                                fwout, pass_i)

                    # ================= splat =================
                    with tc.tile_pool(name="sp", bufs=1) as sp:
                        Fbo = mk_odd(sp, Fb, [P, 4, g.SR, g.SC], "Fbo")
                        vv = sp.tile([P, g.SR, g.SC], F32, tag="vv")
                        uu = sp.tile([P, g.SR, g.SC], F32, tag="uu")
                        load_region(vv[:], fls[1], pass_i, g.SH)
                        load_region(uu[:], fls[0], pass_i, g.SH)
                        nc.vector.memset(acc5[:, 3:5], 0.0)

                        # wX cache for all dx (+ odd-aligned copies)
                        dxs = sorted({dx for (_, dx) in spr})
                        q32 = [sp.tile([P, g.SR, g.SC], F32, tag=f"q32{i}",
                                       name=f"q32{i}") for i in range(2)]
                        m16_ = sp.tile([P, g.SR, g.SC], FP16, tag="m16")
                        m16 = [m16_, m16_]
                        wX, wXo = {}, {}
                        for i, dx in enumerate(dxs):
                            q, m = q32[i % 2], m16[i % 2]
                            t = sp.tile([P, g.SR, g.SC], FP16, tag=f"wX{dx}",
                                        name=f"wX{dx}")
                            act(out=q[:], in_=uu[:], func=AF.Square,
                                scale=GSC, bias=float(-dx) * GSC)
                            act(out=t[:], in_=q[:], func=AF.Exp, scale=-1.0)
                            tss(out=m[:], in_=q[:], scalar=float(QTHR), op=OP.is_lt)
                            tt(out=t[:], in0=t[:], in1=m[:], op=OP.mult)
                            wX[dx] = t
                            if (g.SH - dx) % 2 != 0:
                                wXo[dx] = mk_odd(sp, t, [P, g.SR, g.SC],
                                                 f"wXo{dx}")

                        CH3 = psum_chunks(NP3)
                        with tc.tile_pool(name="pp", bufs=1, space="PSUM") as pp:
                            psumt = pp.tile([P, NP3], F32, tag="ps")
                            cur_dy = None
                            idy = 0
                            wY = wYo = None
                            nspr = len(spr)
                            for pi, (dy, dx) in enumerate(spr):
                                if dy != cur_dy:
                                    q, m = q32[idy % 2], m16[idy % 2]
                                    wY = sp.tile([P, g.SR, g.SC], FP16,
                                                 tag=f"wY{idy % 2}", name=f"wY{idy % 2}")
                                    act(out=q[:], in_=vv[:], func=AF.Square,
                                        scale=GSC, bias=float(-dy) * GSC)
                                    act(out=wY[:], in_=q[:], func=AF.Exp, scale=-1.0)
                                    tss(out=m[:], in_=q[:], scalar=float(QTHR),
                                        op=OP.is_lt)
                                    tt(out=wY[:], in0=wY[:], in1=m[:], op=OP.mult)
                                    wYo = mk_odd(sp, wY, [P, g.SR, g.SC],
                                                 f"wYo{idy % 2}")
                                    idy += 1
                                    cur_dy = dy
                                orr = g.SH - dy
                                occ_ = g.SH - dx
                                if occ_ % 2 == 0:
                                    wYt, wXt, Fbt, oc = wY, wX[dx], Fb, occ_
                                else:
                                    wYt, wXt, Fbt, oc = wYo, wXo[dx], Fbo, occ_ - 1
                                spair = sp.tile([P, g.OR, g.OC], FP16,
                                                tag=f"spair{pi % 2}",
                                                name=f"spair{pi % 2}")
                                tt(out=spair[:],
                                   in0=wYt[:, orr:orr + g.OR, oc:oc + g.OC],
                                   in1=wXt[:, orr:orr + g.OR, oc:oc + g.OC],
                                   op=OP.mult)
                                tmp5 = sp.tile([P, 4, g.OR, g.OC], FP16,
                                               tag=f"tmp5{pi % 3}",
                                               name=f"tmp5{pi % 3}")
                                tt(out=tmp5[:],
                                   in0=Fbt[:, :, orr:orr + g.OR, oc:oc + g.OC],
                                   in1=_bc(spair[:], 4), op=OP.mult)
                                t5 = tmp5[:]
                                for (s, sz, bstart) in CH3:
                                    nc.tensor.matmul(out=psumt[:, s:s + sz],
                                                     lhsT=identt[:],
                                                     rhs=flat_ap(t5, s, sz),
                                                     start=(pi == 0 and bstart),
                                                     stop=(pi == nspr - 1))
                                tt(out=acc5[:, 3], in0=acc5[:, 3],
                                   in1=tmp5[:, 3], op=OP.add)
                                tt(out=acc5[:, 4], in0=acc5[:, 4],
                                   in1=spair[:], op=OP.add)
                            # single full-span evac: depends on (and so waits
                            # for) every matmul group; a per-channel evac
                            # would read a bank TensorE is still writing
                            act(out=acc5[:, 0:3], in_=psumt[:, 0:NP3],
                                func=AF.Copy)

                    for ch in range(5):
                        store_plane(acc5[:, ch], aout[ch], pass_i)

    nc.compile()
    return nc


# ----------------------------------------------------------------------------
# host entry
# ----------------------------------------------------------------------------
def run(input1, input2, input3, input4, NSY=12, NSX=32, npart=128, ncores=8,
        verbose=False, trace=False, S=2.0):
    input1 = np.ascontiguousarray(np.asarray(input1, np.float32))
    input2 = np.ascontiguousarray(np.asarray(input2, np.float32))
    input3 = np.ascontiguousarray(np.asarray(input3, np.float32))
    input4 = np.ascontiguousarray(np.asarray(input4, np.float32))
    B, C, H, W = input1.shape
    halves = ncores // B
    assert B * halves == ncores and H % halves == 0
    H2 = H // halves

    xs1 = np.arange(W, dtype=np.float32)[None, None, :]
    ys1 = np.arange(H, dtype=np.float32)[None, :, None]

    def prep(flow):
        u, v = flow[:, 0], flow[:, 1]
        uq = (xs1 + u) - xs1
        vq = (ys1 + v) - ys1
        Os = (np.abs(uq) >= S) | (np.abs(vq) >= S)
        uw = np.clip(xs1 + u, 0.0, W - 1.0) - xs1
        vw = np.clip(ys1 + v, 0.0, H - 1.0) - ys1
        Ow = (np.abs(uw) >= S) | (np.abs(vw) >= S)
        # ship pre-quantized / pre-clipped flows: device uses them directly
        fww = np.stack([uw, vw], 1)
        fss = np.stack([uq, vq], 1)
        # exact-integer quantized flow would make the symmetric q-mask drop
        # the +2 tap; nudge up an epsilon (floor and window preserved)
        fss = np.where(np.floor(fss) == fss, fss + np.float32(1e-5),
                       fss).astype(np.float32)
        f_warp = np.where(Ow[:, None], np.float32(0.0), fww).astype(np.float32)
        f_splat = np.where(Os[:, None], np.float32(PADFLOW), fss).astype(np.float32)
        f_mask = np.where(Os[:, None], np.float32(0.0), fss).astype(np.float32)
        return f_warp, f_splat, f_mask, Os, Ow

    f1w, f1s, f1m, Os1, Ow1 = prep(input3)
    f2w, f2s, f2m, Os2, Ow2 = prep(input4)

    def warp_corr(img2, flow, Ow):
        bidx, yidx, xidx = np.nonzero(Ow)
        u = flow[bidx, 0, yidx, xidx]
        v = flow[bidx, 1, yidx, xidx]
        gx = np.clip(xidx.astype(np.float32) + u, 0.0, W - 1.0).astype(np.float32)
        gy = np.clip(yidx.astype(np.float32) + v, 0.0, H - 1.0).astype(np.float32)
        x0 = np.floor(gx); y0 = np.floor(gy)
        x1 = np.minimum(x0 + 1.0, W - 1.0); y1 = np.minimum(y0 + 1.0, H - 1.0)
        wx = gx - x0; wy = gy - y0
        x0i = x0.astype(np.int64); x1i = x1.astype(np.int64)
        y0i = y0.astype(np.int64); y1i = y1.astype(np.int64)
        corr = np.zeros((B, 3, H, W), np.float16)
        for c in range(3):
            Ia = img2[bidx, c, y0i, x0i]; Ib = img2[bidx, c, y0i, x1i]
            Ic = img2[bidx, c, y1i, x0i]; Id = img2[bidx, c, y1i, x1i]
            val = (Ia * (1 - wx) * (1 - wy) + Ib * wx * (1 - wy)
                   + Ic * (1 - wx) * wy + Id * wx * wy)
            corr[bidx, c, yidx, xidx] = (val - img2[bidx, c, yidx, xidx]).astype(np.float16)
        return corr

    c1 = warp_corr(input2, input3, Ow1)
    c2 = warp_corr(input1, input4, Ow2)

    sh = 3
    for fl in (f1s, f2s):
        inb = fl != PADFLOW
        k = np.floor(fl[inb])
        if k.size:
            sh = max(sh, int(-(k.min() - 1)), int(k.max() + 2))
    wh = sh - 1

    geo = Geo(H, W, NSY, NSX, npart, SH=sh, WH=wh).finish(H2)

    spr = [_splat_pairs([f1m[b] for b in range(B)], sh),
           _splat_pairs([f2m[b] for b in range(B)], sh)]
    wpr = [_warp_pairs([f1w[b] for b in range(B)], H, W, wh),
           _warp_pairs([f2w[b] for b in range(B)], H, W, wh)]
    if verbose:
        print(f"geo: SH={sh} WH={wh} OR={geo.OR} OC={geo.OC} npass={geo.npass} "
              f"patch={geo.PR}x{geo.PC}")
        print(f"splat pairs: {len(spr[0])}/{len(spr[1])}  "
              f"warp pairs: {len(wpr[0])}/{len(wpr[1])}  "
              f"outliers: {Os1.mean():.3f}/{Os2.mean():.3f}")

    key = (geo.key(), ncores,
           tuple(spr[0]), tuple(spr[1]), tuple(wpr[0]), tuple(wpr[1]))
    if key not in _PROGRAM_CACHE:
        _PROGRAM_CACHE[key] = _build_program(geo, spr, wpr, ncores)
    nc = _PROGRAM_CACHE[key]

    IH = geo.IH
    vm_full = np.pad(np.ones((H, W), np.float32), IH)

    in_maps = []
    for core in range(ncores):
        b, half = divmod(core, halves)
        pd = ((0, 0), (IH, IH), (IH, IH))
        sl = slice(half * H2, half * H2 + H2 + 2 * IH)
        m = {
            "i1": np.pad(input1[b], pd).astype(np.float16),
            "i2": np.pad(input2[b], pd).astype(np.float16),
            "f1w": np.pad(f1w[b], pd, constant_values=PADFLOW),
            "f2w": np.pad(f2w[b], pd, constant_values=PADFLOW),
            "f1s": np.pad(f1s[b], pd, constant_values=PADFLOW),
            "f2s": np.pad(f2s[b], pd, constant_values=PADFLOW),
            "c1": np.pad(c1[b], pd),
            "c2": np.pad(c2[b], pd),
            "vm": vm_full,
            "ident": np.eye(128, dtype=np.float16),
        }
        in_maps.append({k: np.ascontiguousarray(
            v if k == "ident" else (v[..., sl, :] if v.ndim == 3 else v[sl]))
            for k, v in m.items()})

    res = bass_utils.run_bass_kernel_spmd(nc, in_maps,
                                          core_ids=list(range(ncores)),
                                          trace=trace)

    acc = np.empty((2, B, 5, H, W), np.float32)
    fwp = np.empty((2, B, H, W), np.float32)
    for core in range(ncores):
        b, half = divmod(core, halves)
        r = res.results[core]
        sl = slice(half * H2, (half + 1) * H2)
        acc[0, b, :, sl] = r["a1"].astype(np.float32)
        acc[1, b, :, sl] = r["a2"].astype(np.float32)
        fwp[0, b, sl] = r["fw1"].astype(np.float32)
        fwp[1, b, sl] = r["fw2"].astype(np.float32)

    # ---- host fixup: add dropped outlier splat contributions ----
    for br, (flow, Os, img) in enumerate(((input3, Os1, input1),
                                          (input4, Os2, input2))):
        bidx, yidx, xidx = np.nonzero(Os)
        if bidx.size == 0:
            continue
        u = flow[bidx, 0, yidx, xidx]
        v = flow[bidx, 1, yidx, xidx]
        tx = (xidx.astype(np.float32) + u).astype(np.float32)
        ty = (yidx.astype(np.float32) + v).astype(np.float32)
        fx = np.floor(tx); fy = np.floor(ty)
        fwv = fwp[br, bidx, yidx, xidx]
        Fv = np.stack([img[bidx, c, yidx, xidx] * fwv for c in range(3)]
                      + [fwv, np.ones_like(fwv)])          # [5, n]
        boff = bidx * (H * W)
        idx_l, w_l = [], []
        for dy in (-1, 0, 1, 2):
            for dx in (-1, 0, 1, 2):
                ix = fx + dx; iy = fy + dy
                d2 = ((tx - ix) ** 2 + (ty - iy) ** 2).astype(np.float32)
                gw = np.exp(-d2 * np.float32(INV2S2)).astype(np.float32)
                gw = np.where(gw > TAO_R, gw, np.float32(0.0))
                valid = (ix >= 0) & (ix < W) & (iy >= 0) & (iy < H)
                gw = np.where(valid, gw, np.float32(0.0))
                idx = (boff + np.clip(iy, 0, H - 1) * W
                       + np.clip(ix, 0, W - 1)).astype(np.int64)
                idx_l.append(idx)
                w_l.append(gw)
        idx_all = np.concatenate(idx_l)
        w_all = np.concatenate(w_l)
        nrep = len(idx_l)
        for ch in range(5):
            add = np.bincount(idx_all, weights=w_all * np.tile(Fv[ch], nrep),
                              minlength=B * H * W)
            acc[br, :, ch] += add.reshape(B, H, W).astype(np.float32)

    # ---- blend ----
    t = np.float32(THRESH)
    p1, pw1, rw1 = acc[0, :, :3], acc[0, :, 3:4], acc[0, :, 4:5]
    p2, pw2, rw2 = acc[1, :, :3], acc[1, :, 3:4], acc[1, :, 4:5]
    i1b_ = p1 / (pw1 + t)
    w1 = pw1 / (rw1 + t)
    i2b_ = p2 / (pw2 + t)
    w2 = pw2 / (rw2 + t)
    outp = ((i1b_ * w1 + i2b_ * w2) / (w1 + w2 + np.float32(EPS))).astype(np.float32)
    if trace:
        return outp, res
    return outp


def kernel(input1, input2, input3, input4):
    return run(np.asarray(input1), np.asarray(input2),
               np.asarray(input3), np.asarray(input4),
               NSY=12, NSX=32, npart=128, ncores=8)


# revision 30
# speedup vs baseline: 1.1041x; 1.1041x over previous
"""AdaptiveWeightInterpolationModule on 8 Trainium2 NeuronCores, v2.

Forward-splat frame interpolation: two branches, each = bilinear backward
warp (photometric weight fw) + truncated-gaussian forward splat of
(pixel*fw, fw, ones), then adaptive blend.

Strategy v2 (vs baseline)
-------------------------
- Flow-range clipping: pixels whose flow magnitude >= S (=2.0) on either
  axis are outliers (~9% for randn flow).  The device computes the dense
  masked-shift warp/splat only over the small clipped range (splat pairs
  167->~49, warp pairs 115->~25, and halos SH 7->3 so every tile shrinks
  ~1.4x).  Outliers are handled exactly:
    * warp: device samples with flow 0 (identity, weight 1) and adds a
      host-computed correction plane (true bilinear sample - center).
    * splat: outlier sources are dropped on device (PADFLOW) and their
      16-tap contributions are added on the host into the accumulator
      planes, using the device-computed fw plane (exact).
  The device outputs the 5 accumulator planes per branch + fw planes;
  the host applies the fixup and the final (cheap) blend.
- Single q-threshold mask (is_lt on the squared distance) instead of
  is_ge*is_lt per tap axis.
- fp16 4B-alignment: +1-column shifted duplicates of the shifted-read
  tiles (Fb, i2b, wX, wY) so every pair MAC runs in the DVE 2x perf mode
  regardless of shift parity.
- Patch layout as baseline: 128 partitions each own a 30x40 out-tile,
  free dim = flattened patch with halos; data-dependent taps are dense
  masked shift-mul-accs; weights on the Scalar engine.
"""

import numpy as np

import concourse.bass as bass
from concourse import bacc, mybir
from concourse.tile import TileContext
from concourse import bass_utils

F32 = mybir.dt.float32
FP16 = mybir.dt.float16
AF = mybir.ActivationFunctionType
OP = mybir.AluOpType

SIGMA_D = 1.5
TAO_R = 0.05
LAMBDA_E = 30.0 / 255.0
THRESH = 1e-6
EPS = 1e-6
PADFLOW = 30000.0
GSC = 1.0 / (SIGMA_D * np.sqrt(2.0))
QTHR = 4.0 * GSC * GSC          # window mask threshold on ((d)*gsc)^2
INV2S2 = 1.0 / (2.0 * SIGMA_D * SIGMA_D)

_PROGRAM_CACHE = {}


class Geo:
    def __init__(self, H, W, NSY, NSX, npart, SH, WH):
        self.H, self.W = H, W
        self.NSY, self.NSX = NSY, NSX
        assert W % NSX == 0
        self.OC = W // NSX
        self.npart = npart
        assert (NSY * NSX) % npart == 0
        self.npass = (NSY * NSX) // npart
        assert npart % NSX == 0
        self.SYPP = npart // NSX
        self.SH = SH
        self.EH = SH + 1
        self.WH = WH
        self.IH = self.EH + WH

    def finish(self, H2):
        assert H2 % self.NSY == 0
        self.H2 = H2
        self.OR = H2 // self.NSY
        self.PR = self.OR + 2 * self.IH
        self.PC = self.OC + 2 * self.IH
        self.ER = self.OR + 2 * self.EH
        self.EC = self.OC + 2 * self.EH
        self.SR = self.OR + 2 * self.SH
        self.SC = self.OC + 2 * self.SH
        self.Rpad = H2 + 2 * self.IH
        self.Cpad = self.W + 2 * self.IH
        return self

    def key(self):
        return (self.H, self.W, self.NSY, self.NSX, self.npart, self.SH,
                self.WH, self.H2)


def _splat_pairs(flow_list, sh):
    occ = set()
    for fl in flow_list:
        H, W = fl.shape[1], fl.shape[2]
        ys = np.arange(H, dtype=np.float32)[:, None]
        xs = np.arange(W, dtype=np.float32)[None, :]
        ueff = (xs + fl[0]) - xs
        veff = (ys + fl[1]) - ys
        ku = np.floor(ueff).astype(np.int64).ravel() + 4096
        kv = np.floor(veff).astype(np.int64).ravel() + 4096
        occ.update(np.unique(kv * 8192 + ku).tolist())
    pairs = set()
    for o in occ:
        a, b = divmod(o, 8192)
        a -= 4096
        b -= 4096
        for d in (-1, 0, 1, 2):
            for e in (-1, 0, 1, 2):
                pairs.add((a + d, b + e))
    pairs = sorted(pairs)
    for (dy, dx) in pairs:
        assert abs(dy) <= sh and abs(dx) <= sh, (dy, dx, sh)
    return pairs


def _warp_pairs(flow_list, H, W, wh):
    occ = set()
    for fl in flow_list:
        u, v = fl[0], fl[1]
        ys = np.arange(H, dtype=np.float32)[:, None]
        xs = np.arange(W, dtype=np.float32)[None, :]
        kv = np.floor(np.clip(ys + v, 0.0, H - 1.0) - ys).astype(np.int64).ravel() + 4096
        ku = np.floor(np.clip(xs + u, 0.0, W - 1.0) - xs).astype(np.int64).ravel() + 4096
        occ.update(np.unique(kv * 8192 + ku).tolist())
    pairs = set()
    for o in occ:
        a, b = divmod(o, 8192)
        a -= 4096
        b -= 4096
        for d in (0, 1):
            for e in (0, 1):
                pairs.add((a + d, b + e))
    pairs = sorted(pairs)
    for (ky, kx) in pairs:
        assert abs(ky) <= wh and abs(kx) <= wh, (ky, kx, wh)
    return pairs


def _bc(ap, n):
    """Insert a broadcast (step-0) dim of size n after the partition dim."""
    return bass.AP(ap.tensor, ap.offset, [ap.ap[0], [0, n]] + list(ap.ap[1:]))


def _build_program(geo, splat_pairs, warp_pairs, num_devices):
    g = geo
    nc = bacc.Bacc("TRN2", target_bir_lowering=False, debug=False,
                   num_devices=num_devices)

    # pre-register activation bias constants
    biases = {1.0}
    for wpr_ in warp_pairs:
        for (ky, kx) in wpr_:
            biases.add(float(-ky))
            biases.add(float(-kx))
    for spr_ in splat_pairs:
        for (dy, dx) in spr_:
            biases.add(float(-dy) * GSC)
            biases.add(float(-dx) * GSC)
    for i, val in enumerate(sorted(biases)):
        key = (F32, val)
        if key not in nc.const_aps.aps:
            t = nc.alloc_sbuf_tensor(f"constap{i}", [128, 1], F32)
            nc.gpsimd.memset(t.ap(), val)
            nc.const_aps.aps[key] = t.ap()
    nc.all_engine_barrier()

    dr = {}
    for nm in ("i1", "i2"):
        dr[nm] = nc.dram_tensor(nm, [3, g.Rpad, g.Cpad], FP16, kind="ExternalInput").ap()
    for nm in ("f1w", "f2w", "f1s", "f2s"):
        dr[nm] = nc.dram_tensor(nm, [2, g.Rpad, g.Cpad], F32, kind="ExternalInput").ap()
    for nm in ("c1", "c2"):
        dr[nm] = nc.dram_tensor(nm, [3, g.Rpad, g.Cpad], FP16, kind="ExternalInput").ap()
    for nm in ("vm",):
        dr[nm] = nc.dram_tensor(nm, [g.Rpad, g.Cpad], F32, kind="ExternalInput").ap()
    dr["ident"] = nc.dram_tensor("ident", [128, 128], FP16,
                                 kind="ExternalInput").ap()
    outs = {
        "a1": nc.dram_tensor("a1", [5, g.H2, g.W], FP16, kind="ExternalOutput").ap(),
        "a2": nc.dram_tensor("a2", [5, g.H2, g.W], FP16, kind="ExternalOutput").ap(),
        "fw1": nc.dram_tensor("fw1", [g.H2, g.W], FP16, kind="ExternalOutput").ap(),
        "fw2": nc.dram_tensor("fw2", [g.H2, g.W], FP16, kind="ExternalOutput").ap(),
    }

    P = g.npart

    def load_region(tile_ap, base2d, pass_i, rh):
        nr = g.OR + 2 * rh
        nc_ = g.OC + 2 * rh
        off = (base2d.offset
               + (pass_i * g.SYPP * g.OR + g.IH - rh) * g.Cpad
               + (g.IH - rh))
        for sy in range(g.SYPP):
            ap = bass.AP(base2d.tensor, off + sy * g.OR * g.Cpad,
                         [[g.OC, g.NSX], [g.Cpad, nr], [1, nc_]])
            nc.sync.dma_start(out=tile_ap[sy * g.NSX:(sy + 1) * g.NSX], in_=ap)

    def store_plane(tile_ap, base2d, pass_i):
        off = base2d.offset + pass_i * g.SYPP * g.OR * g.W
        for sy in range(g.SYPP):
            ap = bass.AP(base2d.tensor, off + sy * g.OR * g.W,
                         [[g.OC, g.NSX], [g.W, g.OR], [1, g.OC]])
            nc.sync.dma_start(out=ap, in_=tile_ap[sy * g.NSX:(sy + 1) * g.NSX])

    tt = nc.vector.tensor_tensor
    tss = nc.vector.tensor_single_scalar
    act = nc.scalar.activation

    def mk_odd(pool, src, shape, tag):
        """+1-column copy so odd free-dim shifts read 4B-aligned."""
        t = pool.tile(shape, FP16, tag=tag, name=tag)
        sl_in = tuple([slice(None)] * (len(shape) - 1) + [slice(1, shape[-1])])
        sl_out = tuple([slice(None)] * (len(shape) - 1) + [slice(0, shape[-1] - 1)])
        nc.vector.tensor_copy(out=t[sl_out], in_=src[sl_in])
        return t

    def flat_ap(ap, s, n):
        return bass.AP(ap.tensor, ap.offset + s, [ap.ap[0], [1, n]])

    # ragged bank-aligned PSUM chunking for nfp32 contiguous accumulators
    def psum_chunks(ntot):
        chunks = []
        flat = 0
        while flat < ntot:
            room = 512 - (flat % 512)
            sz = min(512, ntot - flat, room)
            chunks.append((flat, sz, flat % 512 == 0))
            flat += sz
        return chunks

    NP3 = 3 * 30 * 40  # 3 fp32 accumulator channels, must fit 8 PSUM banks
    assert 3 * g.OR * g.OC == NP3 and NP3 <= 4096

    with TileContext(nc) as tc:
      with tc.tile_pool(name="sing", bufs=1) as sing:
        identt = sing.tile([128, 128], FP16, tag="id")
        nc.sync.dma_start(out=identt[:], in_=dr["ident"])
        for pass_i in range(g.npass):
            for br in range(2):
                imA = dr["i1"] if br == 0 else dr["i2"]
                imB = dr["i2"] if br == 0 else dr["i1"]
                flw = dr["f1w"] if br == 0 else dr["f2w"]
                fls = dr["f1s"] if br == 0 else dr["f2s"]
                crr = dr["c1"] if br == 0 else dr["c2"]
                aout = outs["a1"] if br == 0 else outs["a2"]
                fwout = outs["fw1"] if br == 0 else outs["fw2"]
                spr = splat_pairs[br]
                wpr = warp_pairs[br]

                with tc.tile_pool(name="brp", bufs=1) as brp:
                    errt = brp.tile([P, g.ER, g.EC], F32, tag="err")
                    i1b = brp.tile([P, 3, g.ER, g.EC], FP16, tag="i1b")
                    acc5 = brp.tile([P, 5, g.OR, g.OC], FP16, tag="acc5")

                    # ================= warp + err =================
                    with tc.tile_pool(name="wp", bufs=1) as wp:
                        for c in range(3):
                            load_region(i1b[:, c], imA[c], pass_i, g.EH)
                        vp = wp.tile([P, g.ER, g.EC], F32, tag="vp")
                        up = wp.tile([P, g.ER, g.EC], F32, tag="up")
                        ct = wp.tile([P, g.ER, g.EC], F32, tag="ct")
                        load_region(vp[:], flw[1], pass_i, g.EH)
                        load_region(up[:], flw[0], pass_i, g.EH)
                        load_region(ct[:], dr["vm"], pass_i, g.EH)

                        i2b = wp.tile([P, 3, g.PR, g.PC], FP16, tag="i2b")
                        for c in range(3):
                            load_region(i2b[:, c], imB[c], pass_i, g.IH)
                        i2bo = mk_odd(wp, i2b, [P, 3, g.PR, g.PC], "i2bo")

                        # per-kx / per-ky triangle weights (Scalar engine)
                        kxs = sorted({kx for (_, kx) in wpr})
                        kys = sorted({ky for (ky, _) in wpr})
                        trix = {}
                        for kx in kxs:
                            t = wp.tile([P, g.ER, g.EC], FP16, tag=f"trix{kx}",
                                        name=f"trix{kx}")
                            act(out=t[:], in_=up[:], func=AF.Abs, bias=float(-kx))
                            act(out=t[:], in_=t[:], func=AF.Relu, scale=-1.0, bias=1.0)
                            trix[kx] = t
                        triy = {}
                        for ky in kys:
                            t = wp.tile([P, g.ER, g.EC], FP16, tag=f"triy{ky}",
                                        name=f"triy{ky}")
                            act(out=t[:], in_=vp[:], func=AF.Abs, bias=float(-ky))
                            act(out=t[:], in_=t[:], func=AF.Relu, scale=-1.0, bias=1.0)
                            triy[ky] = t

                        wacc = wp.tile([P, 3, g.ER, g.EC], FP16, tag="wacc")
                        # corr planes seed the accumulator (outlier fixup);
                        # staged copy: never DMA into a tile that is then RMW'd
                        ldh = wp.tile([P, 3, g.ER, g.EC], FP16, tag="ldh")
                        for c in range(3):
                            load_region(ldh[:, c], crr[c], pass_i, g.EH)
                        nc.vector.tensor_copy(out=wacc[:, 2], in_=ldh[:, 2])

                        NPW = 2 * g.ER * g.EC
                        assert NPW <= 4096
                        CHW = psum_chunks(NPW)
                        d0 = g.IH - g.EH
                        with tc.tile_pool(name="wpp", bufs=1, space="PSUM") as wpp:
                            psw = wpp.tile([P, NPW], F32, tag="pw")
                            # seed PSUM chs 0-1 with the corr planes
                            l2 = ldh[:]
                            for (s, sz, bstart) in CHW:
                                nc.tensor.matmul(out=psw[:, s:s + sz],
                                                 lhsT=identt[:],
                                                 rhs=flat_ap(l2, s, sz),
                                                 start=bstart, stop=False)
                            nwpr = len(wpr)
                            for pi, (ky, kx) in enumerate(wpr):
                                wpair = wp.tile([P, g.ER, g.EC], FP16, tag="wpair")
                                tt(out=wpair[:], in0=triy[ky][:], in1=trix[kx][:],
                                   op=OP.mult)
                                oc = d0 + kx
                                src = i2b if oc % 2 == 0 else i2bo
                                if oc % 2 != 0:
                                    oc -= 1
                                tmp3 = wp.tile([P, 3, g.ER, g.EC], FP16,
                                               tag=f"tmp3{pi % 3}",
                                               name=f"tmp3{pi % 3}")
                                tt(out=tmp3[:],
                                   in0=src[:, :, d0 + ky:d0 + ky + g.ER, oc:oc + g.EC],
                                   in1=_bc(wpair[:], 3), op=OP.mult)
                                t3 = tmp3[:]
                                for (s, sz, bstart) in CHW:
                                    nc.tensor.matmul(out=psw[:, s:s + sz],
                                                     lhsT=identt[:],
                                                     rhs=flat_ap(t3, s, sz),
                                                     start=False,
                                                     stop=(pi == nwpr - 1))
                                tt(out=wacc[:, 2], in0=wacc[:, 2],
                                   in1=tmp3[:, 2], op=OP.add)
                            act(out=wacc[:, 0:2], in_=psw[:, 0:NPW],
                                func=AF.Copy)

                        # err = (sum_c |i1_c - wacc_c|) * vmask
                        d3 = wp.tile([P, 3, g.ER, g.EC], FP16, tag="tmp30")
                        tt(out=d3[:], in0=i1b[:], in1=wacc[:], op=OP.subtract)
                        act(out=d3[:], in_=d3[:], func=AF.Abs)
                        tt(out=errt[:], in0=d3[:, 0], in1=d3[:, 1], op=OP.add)
                        tt(out=errt[:], in0=errt[:], in1=d3[:, 2], op=OP.add)
                        tt(out=errt[:], in0=errt[:], in1=ct[:], op=OP.mult)

                    # ================= blur -> fw -> F =================
                    Fb = brp.tile([P, 4, g.SR, g.SC], FP16, tag="Fb")
                    with tc.tile_pool(name="bp", bufs=1) as bp:
                        d1 = g.EH - g.SH  # = 1
                        tmpb = bp.tile([P, g.ER, g.SC], F32, tag="tmpb")
                        tt(out=tmpb[:], in0=errt[:, :, d1 - 1:d1 - 1 + g.SC],
                           in1=errt[:, :, d1 + 1:d1 + 1 + g.SC], op=OP.add)
                        tt(out=tmpb[:], in0=tmpb[:],
                           in1=errt[:, :, d1:d1 + g.SC], op=OP.add)
                        blur = bp.tile([P, g.SR, g.SC], F32, tag="blur")
                        tt(out=blur[:], in0=tmpb[:, d1 - 1:d1 - 1 + g.SR, :],
                           in1=tmpb[:, d1 + 1:d1 + 1 + g.SR, :], op=OP.add)
                        tt(out=blur[:], in0=blur[:],
                           in1=tmpb[:, d1:d1 + g.SR, :], op=OP.add)
                        s = 1.0 / (27.0 * LAMBDA_E)
                        act(out=blur[:], in_=blur[:], func=AF.Square, scale=float(s))
                        act(out=Fb[:, 3], in_=blur[:], func=AF.Exp, scale=-1.0)
                        for c in range(3):
                            tt(out=Fb[:, c], in0=i1b[:, c, d1:d1 + g.SR, d1:d1 + g.SC],
                               in1=Fb[:, 3], op=OP.mult)
                    store_plane(Fb[:, 3, g.SH:g.SH + g.OR, g.SH:g.SH + g.OC],
                                fwout, pass_i)

                    # ================= splat =================
                    with tc.tile_pool(name="sp", bufs=1) as sp:
                        Fbo = mk_odd(sp, Fb, [P, 4, g.SR, g.SC], "Fbo")
                        vv = sp.tile([P, g.SR, g.SC], F32, tag="vv")
                        uu = sp.tile([P, g.SR, g.SC], F32, tag="uu")
                        load_region(vv[:], fls[1], pass_i, g.SH)
                        load_region(uu[:], fls[0], pass_i, g.SH)
                        nc.vector.memset(acc5[:, 3:5], 0.0)

                        # wX cache for all dx (+ odd-aligned copies)
                        dxs = sorted({dx for (_, dx) in spr})
                        q32 = [sp.tile([P, g.SR, g.SC], F32, tag=f"q32{i}",
                                       name=f"q32{i}") for i in range(2)]
                        m16_ = sp.tile([P, g.SR, g.SC], FP16, tag="m16")
                        m16 = [m16_, m16_]
                        wX, wXo = {}, {}
                        for i, dx in enumerate(dxs):
                            q, m = q32[i % 2], m16[i % 2]
                            t = sp.tile([P, g.SR, g.SC], FP16, tag=f"wX{dx}",
                                        name=f"wX{dx}")
                            act(out=q[:], in_=uu[:], func=AF.Square,
                                scale=GSC, bias=float(-dx) * GSC)
                            act(out=t[:], in_=q[:], func=AF.Exp, scale=-1.0)
                            tss(out=m[:], in_=q[:], scalar=float(QTHR), op=OP.is_lt)
                            tt(out=t[:], in0=t[:], in1=m[:], op=OP.mult)
                            wX[dx] = t
                            if (g.SH - dx) % 2 != 0:
                                wXo[dx] = mk_odd(sp, t, [P, g.SR, g.SC],
                                                 f"wXo{dx}")

                        CH3 = psum_chunks(NP3)
                        with tc.tile_pool(name="pp", bufs=1, space="PSUM") as pp:
                            psumt = pp.tile([P, NP3], F32, tag="ps")
                            cur_dy = None
                            idy = 0
                            wY = wYo = None
                            nspr = len(spr)
                            for pi, (dy, dx) in enumerate(spr):
                                if dy != cur_dy:
                                    q, m = q32[idy % 2], m16[idy % 2]
                                    wY = sp.tile([P, g.SR, g.SC], FP16,
                                                 tag=f"wY{idy % 2}", name=f"wY{idy % 2}")
                                    act(out=q[:], in_=vv[:], func=AF.Square,
                                        scale=GSC, bias=float(-dy) * GSC)
                                    act(out=wY[:], in_=q[:], func=AF.Exp, scale=-1.0)
                                    tss(out=m[:], in_=q[:], scalar=float(QTHR),
                                        op=OP.is_lt)
                                    tt(out=wY[:], in0=wY[:], in1=m[:], op=OP.mult)
                                    wYo = mk_odd(sp, wY, [P, g.SR, g.SC],
                                                 f"wYo{idy % 2}")
                                    idy += 1
                                    cur_dy = dy
                                orr = g.SH - dy
                                occ_ = g.SH - dx
                                if occ_ % 2 == 0:
                                    wYt, wXt, Fbt, oc = wY, wX[dx], Fb, occ_
                                else:
                                    wYt, wXt, Fbt, oc = wYo, wXo[dx], Fbo, occ_ - 1
                                spair = sp.tile([P, g.OR, g.OC], FP16,
                                                tag=f"spair{pi % 2}",
                                                name=f"spair{pi % 2}")
                                tt(out=spair[:],
                                   in0=wYt[:, orr:orr + g.OR, oc:oc + g.OC],
                                   in1=wXt[:, orr:orr + g.OR, oc:oc + g.OC],
                                   op=OP.mult)
                                tmp5 = sp.tile([P, 4, g.OR, g.OC], FP16,
                                               tag=f"tmp5{pi % 3}",
                                               name=f"tmp5{pi % 3}")
                                tt(out=tmp5[:],
                                   in0=Fbt[:, :, orr:orr + g.OR, oc:oc + g.OC],
                                   in1=_bc(spair[:], 4), op=OP.mult)
                                t5 = tmp5[:]
                                for (s, sz, bstart) in CH3:
                                    nc.tensor.matmul(out=psumt[:, s:s + sz],
                                                     lhsT=identt[:],
                                                     rhs=flat_ap(t5, s, sz),
                                                     start=(pi == 0 and bstart),
                                                     stop=(pi == nspr - 1))
                                tt(out=acc5[:, 3], in0=acc5[:, 3],
                                   in1=tmp5[:, 3], op=OP.add)
                                tt(out=acc5[:, 4], in0=acc5[:, 4],
                                   in1=spair[:], op=OP.add)
                            # single full-span evac: depends on (and so waits
                            # for) every matmul group; a per-channel evac
                            # would read a bank TensorE is still writing
                            act(out=acc5[:, 0:3], in_=psumt[:, 0:NP3],
                                func=AF.Copy)

                    for ch in range(5):
                        store_plane(acc5[:, ch], aout[ch], pass_i)

    nc.compile()
    return nc


# ----------------------------------------------------------------------------
# host entry
# ----------------------------------------------------------------------------
def run(input1, input2, input3, input4, NSY=12, NSX=32, npart=128, ncores=8,
        verbose=False, trace=False, S=2.0):
    input1 = np.ascontiguousarray(np.asarray(input1, np.float32))
    input2 = np.ascontiguousarray(np.asarray(input2, np.float32))
    input3 = np.ascontiguousarray(np.asarray(input3, np.float32))
    input4 = np.ascontiguousarray(np.asarray(input4, np.float32))
    B, C, H, W = input1.shape
    halves = ncores // B
    assert B * halves == ncores and H % halves == 0
    H2 = H // halves

    xs1 = np.arange(W, dtype=np.float32)[None, None, :]
    ys1 = np.arange(H, dtype=np.float32)[None, :, None]

    def prep(flow):
        u, v = flow[:, 0], flow[:, 1]
        uq = (xs1 + u) - xs1
        vq = (ys1 + v) - ys1
        Os = (np.abs(uq) >= S) | (np.abs(vq) >= S)
        uw = np.clip(xs1 + u, 0.0, W - 1.0) - xs1
        vw = np.clip(ys1 + v, 0.0, H - 1.0) - ys1
        Ow = (np.abs(uw) >= S) | (np.abs(vw) >= S)
        # ship pre-quantized / pre-clipped flows: device uses them directly
        fww = np.stack([uw, vw], 1)
        fss = np.stack([uq, vq], 1)
        # exact-integer quantized flow would make the symmetric q-mask drop
        # the +2 tap; nudge up an epsilon (floor and window preserved)
        fss = np.where(np.floor(fss) == fss, fss + np.float32(1e-5),
                       fss).astype(np.float32)
        f_warp = np.where(Ow[:, None], np.float32(0.0), fww).astype(np.float32)
        f_splat = np.where(Os[:, None], np.float32(PADFLOW), fss).astype(np.float32)
        f_mask = np.where(Os[:, None], np.float32(0.0), fss).astype(np.float32)
        return f_warp, f_splat, f_mask, Os, Ow

    f1w, f1s, f1m, Os1, Ow1 = prep(input3)
    f2w, f2s, f2m, Os2, Ow2 = prep(input4)

    def warp_corr(img2, flow, Ow):
        bidx, yidx, xidx = np.nonzero(Ow)
        u = flow[bidx, 0, yidx, xidx]
        v = flow[bidx, 1, yidx, xidx]
        gx = np.clip(xidx.astype(np.float32) + u, 0.0, W - 1.0).astype(np.float32)
        gy = np.clip(yidx.astype(np.float32) + v, 0.0, H - 1.0).astype(np.float32)
        x0 = np.floor(gx); y0 = np.floor(gy)
        x1 = np.minimum(x0 + 1.0, W - 1.0); y1 = np.minimum(y0 + 1.0, H - 1.0)
        wx = gx - x0; wy = gy - y0
        x0i = x0.astype(np.int64); x1i = x1.astype(np.int64)
        y0i = y0.astype(np.int64); y1i = y1.astype(np.int64)
        corr = np.zeros((B, 3, H, W), np.float16)
        for c in range(3):
            Ia = img2[bidx, c, y0i, x0i]; Ib = img2[bidx, c, y0i, x1i]
            Ic = img2[bidx, c, y1i, x0i]; Id = img2[bidx, c, y1i, x1i]
            val = (Ia * (1 - wx) * (1 - wy) + Ib * wx * (1 - wy)
                   + Ic * (1 - wx) * wy + Id * wx * wy)
            corr[bidx, c, yidx, xidx] = (val - img2[bidx, c, yidx, xidx]).astype(np.float16)
        return corr

    c1 = warp_corr(input2, input3, Ow1)
    c2 = warp_corr(input1, input4, Ow2)

    sh = 3
    for fl in (f1s, f2s):
        inb = fl != PADFLOW
        k = np.floor(fl[inb])
        if k.size:
            sh = max(sh, int(-(k.min() - 1)), int(k.max() + 2))
    wh = sh - 1

    geo = Geo(H, W, NSY, NSX, npart, SH=sh, WH=wh).finish(H2)

    spr = [_splat_pairs([f1m[b] for b in range(B)], sh),
           _splat_pairs([f2m[b] for b in range(B)], sh)]
    wpr = [_warp_pairs([f1w[b] for b in range(B)], H, W, wh),
           _warp_pairs([f2w[b] for b in range(B)], H, W, wh)]
    if verbose:
        print(f"geo: SH={sh} WH={wh} OR={geo.OR} OC={geo.OC} npass={geo.npass} "
              f"patch={geo.PR}x{geo.PC}")
        print(f"splat pairs: {len(spr[0])}/{len(spr[1])}  "
              f"warp pairs: {len(wpr[0])}/{len(wpr[1])}  "
              f"outliers: {Os1.mean():.3f}/{Os2.mean():.3f}")

    key = (geo.key(), ncores,
           tuple(spr[0]), tuple(spr[1]), tuple(wpr[0]), tuple(wpr[1]))
    if key not in _PROGRAM_CACHE:
        _PROGRAM_CACHE[key] = _build_program(geo, spr, wpr, ncores)
    nc = _PROGRAM_CACHE[key]

    IH = geo.IH
    vm_full = np.pad(np.ones((H, W), np.float32), IH)

    in_maps = []
    for core in range(ncores):
        b, half = divmod(core, halves)
        pd = ((0, 0), (IH, IH), (IH, IH))
        sl = slice(half * H2, half * H2 + H2 + 2 * IH)
        m = {
            "i1": np.pad(input1[b], pd).astype(np.float16),
            "i2": np.pad(input2[b], pd).astype(np.float16),
            "f1w": np.pad(f1w[b], pd, constant_values=PADFLOW),
            "f2w": np.pad(f2w[b], pd, constant_values=PADFLOW),
            "f1s": np.pad(f1s[b], pd, constant_values=PADFLOW),
            "f2s": np.pad(f2s[b], pd, constant_values=PADFLOW),
            "c1": np.pad(c1[b], pd),
            "c2": np.pad(c2[b], pd),
            "vm": vm_full,
            "ident": np.eye(128, dtype=np.float16),
        }
        in_maps.append({k: np.ascontiguousarray(
            v if k == "ident" else (v[..., sl, :] if v.ndim == 3 else v[sl]))
            for k, v in m.items()})

    res = bass_utils.run_bass_kernel_spmd(nc, in_maps,
                                          core_ids=list(range(ncores)),
                                          trace=trace)

    acc = np.empty((2, B, 5, H, W), np.float32)
    fwp = np.empty((2, B, H, W), np.float32)
    for core in range(ncores):
        b, half = divmod(core, halves)
        r = res.results[core]
        sl = slice(half * H2, (half + 1) * H2)
        acc[0, b, :, sl] = r["a1"].astype(np.float32)
        acc[1, b, :, sl] = r["a2"].astype(np.float32)
        fwp[0, b, sl] = r["fw1"].astype(np.float32)
        fwp[1, b, sl] = r["fw2"].astype(np.float32)

    # ---- host fixup: add dropped outlier splat contributions ----
    for br, (flow, Os, img) in enumerate(((input3, Os1, input1),
                                          (input4, Os2, input2))):
        bidx, yidx, xidx = np.nonzero(Os)
        if bidx.size == 0:
            continue
        u = flow[bidx, 0, yidx, xidx]
        v = flow[bidx, 1, yidx, xidx]
        tx = (xidx.astype(np.float32) + u).astype(np.float32)
        ty = (yidx.astype(np.float32) + v).astype(np.float32)
        fx = np.floor(tx); fy = np.floor(ty)
        fwv = fwp[br, bidx, yidx, xidx]
        Fv = np.stack([img[bidx, c, yidx, xidx] * fwv for c in range(3)]
                      + [fwv, np.ones_like(fwv)])          # [5, n]
        boff = bidx * (H * W)
        idx_l, w_l = [], []
        for dy in (-1, 0, 1, 2):
            for dx in (-1, 0, 1, 2):
                ix = fx + dx; iy = fy + dy
                d2 = ((tx - ix) ** 2 + (ty - iy) ** 2).astype(np.float32)
                gw = np.exp(-d2 * np.float32(INV2S2)).astype(np.float32)
                gw = np.where(gw > TAO_R, gw, np.float32(0.0))
                valid = (ix >= 0) & (ix < W) & (iy >= 0) & (iy < H)
                gw = np.where(valid, gw, np.float32(0.0))
                idx = (boff + np.clip(iy, 0, H - 1) * W
                       + np.clip(ix, 0, W - 1)).astype(np.int64)
                idx_l.append(idx)
                w_l.append(gw)
        idx_all = np.concatenate(idx_l)
        w_all = np.concatenate(w_l)
        nrep = len(idx_l)
        for ch in range(5):
            add = np.bincount(idx_all, weights=w_all * np.tile(Fv[ch], nrep),
                              minlength=B * H * W)
            acc[br, :, ch] += add.reshape(B, H, W).astype(np.float32)

    # ---- blend ----
    t = np.float32(THRESH)
    p1, pw1, rw1 = acc[0, :, :3], acc[0, :, 3:4], acc[0, :, 4:5]
    p2, pw2, rw2 = acc[1, :, :3], acc[1, :, 3:4], acc[1, :, 4:5]
    i1b_ = p1 / (pw1 + t)
    w1 = pw1 / (rw1 + t)
    i2b_ = p2 / (pw2 + t)
    w2 = pw2 / (rw2 + t)
    outp = ((i1b_ * w1 + i2b_ * w2) / (w1 + w2 + np.float32(EPS))).astype(np.float32)
    if trace:
        return outp, res
    return outp


def kernel(input1, input2, input3, input4):
    return run(np.asarray(input1), np.asarray(input2),
               np.asarray(input3), np.asarray(input4),
               NSY=12, NSX=32, npart=128, ncores=8)
